# revision 23
# baseline (speedup 1.0000x reference)
# Bidirectional cross-attention Trainium2 kernel (Bass/Tile), 8-core head-parallel.
#
# Sharding: 16 heads / 8 cores = 2 heads per core (tensor parallel on h); each
# core computes its heads' projections, similarity, both softmax directions and
# its row-parallel partial of the final projections; host sums partials + bias.
#
# Design (final):
#  - everything 2-byte fp16 (fp8 is numerically dead here: in attention's
#    incoherent random sums, per-element quantization error does not average
#    away, so e4m3's ~6% element error would land at the output)
#  - NO DMA-crossbar transposes: sim is computed in BOTH orientations
#    directly on the PE from the same persistent qkT/cqkT operands (swap
#    matmul lhsT/rhs roles). The [i-part, j] exp tiles feed the ctx-side
#    accumulator H = v^T E, the [j-part, i] tiles feed the out-side
#    G = cv^T E^T. Costs one extra sim sweep (PE) + one extra exp sweep
#    (ACT) but removes 16.8 MB/core of 2-byte-element scattered DMA that
#    dominated the old kernel's runtime (~3x the cost model's estimate).
#  - phases = (head, side, seq-half): 16 sim psum tiles + 16 exps each; a
#    phase's H/G accumulation + normalization drains lagged-by-one-phase
#    (sprinkled between the next phase's emissions) so its exp deps are long
#    satisfied and the PE never blocks on ACT (fine-grained PE<-ACT chaining
#    costs ~0.4us/iteration in semaphore/pipeline restarts)
#  - softmax sums ride along as a ones-column in the V operands;
#    normalization: DVE reciprocal (f32r) + gpsimd partition_broadcast (the
#    old K=1 PE ones-broadcast matmuls also coupled the sim psum-pool
#    rotation to the slow norm chain: moving them to the idle Pool engine
#    was worth ~41us/body on HW, 3x the model's estimate)
#  - prologue: chunk-group 0 of the projections runs kt-chased under the
#    input DMA stream (8 psum banks); chunk-group 1 drains INSIDE phase 0
#    through a coexisting 4-bank pool (qk/cqk chunks first so phase 0's
#    later i-blocks find their lhsT ready); PE p-state + Exp-table warmups
#    run during the DMA lead-in
#  - final projections per output half are queued as soon as both heads'
#    accumulations for that half are in the FIFO; only the very last
#    accumulation + 8 final blocks trail the last exp, with their psum
#    copies alternating ACT/DVE (ACT is idle by then)
#  - engine balance per core (cost model): PE ~161us busy, ACT ~143, DVE
#    ~74; ~227us total vs ~162us PE floor
#  - reps>1 builds the same body repeatedly (per-rep pools) for in-NEFF
#    repetition timing: body ~= (t(repsR) - t(reps1)) / (R - 1)

import os
import sys

for _p in ("/opt/trn_rl_repo", "/root/.axon_site/_ro/trn_rl_repo"):
    if os.path.isdir(_p) and _p not in sys.path:
        sys.path.insert(0, _p)

import numpy as np

SEQ_MODE = os.environ.get("KSEQ", "0") == "1"

HEADS = 16
DIM_HEAD = 64
DIM = 1024
SEQ = 2048
N_CORES = 8
HPC = HEADS // N_CORES          # heads per core = 2
FPC = HPC * DIM_HEAD            # feature cols per core = 128
SCALE = DIM_HEAD ** -0.5


def _ts(i, size):
    return slice(i * size, (i + 1) * size)


def build_bass(seq=SEQ, dim=DIM, fpc=FPC, hpc=HPC, num_devices=N_CORES,
               stage='full', reps=1):
    import concourse.bacc as bacc
    import concourse.tile as tile
    import concourse.mybir as mybir
    from contextlib import ExitStack
    from collections import deque

    f32 = mybir.dt.float32
    f16 = mybir.dt.float16
    f32r = mybir.dt.float32r
    Exp = mybir.ActivationFunctionType.Exp

    P = 128
    KT = dim // P              # contraction tiles over DIM (8)
    NT = seq // P              # 128-blocks along sequence (16)
    NCH = seq // 512           # 512-chunks along sequence (4)
    J2 = min(1024, seq)        # exp-tile width
    N2CH = seq // J2           # seq halves (2)
    HPT = J2 // 512            # 512-chunks per half (2)
    OCH = dim // 512           # 512-chunks of output dim (2)
    dh = DIM_HEAD
    vw = dh + 1

    nc = bacc.Bacc("TRN2", target_bir_lowering=False, debug=False,
                   num_devices=num_devices)

    xT = nc.dram_tensor("xT", (dim, seq), f16, kind="ExternalInput").ap()
    cT = nc.dram_tensor("cT", (dim, seq), f16, kind="ExternalInput").ap()
    wqk = nc.dram_tensor("wqk", (dim, fpc), f16, kind="ExternalInput").ap()
    wv = nc.dram_tensor("wv", (dim, fpc), f16, kind="ExternalInput").ap()
    wcqk = nc.dram_tensor("wcqk", (dim, fpc), f16, kind="ExternalInput").ap()
    wcv = nc.dram_tensor("wcv", (dim, fpc), f16, kind="ExternalInput").ap()
    wout = nc.dram_tensor("wout", (fpc, dim), f16, kind="ExternalInput").ap()
    wcout = nc.dram_tensor("wcout", (fpc, dim), f16, kind="ExternalInput").ap()
    out_p = nc.dram_tensor("out_p", (seq, dim), f16, kind="ExternalOutput").ap()
    ctx_p = nc.dram_tensor("ctx_p", (seq, dim), f16, kind="ExternalOutput").ap()

    out_view = out_p.rearrange("(ib p) o -> p ib o", p=P)
    ctx_view = ctx_p.rearrange("(ib p) o -> p ib o", p=P)

    with tile.TileContext(nc) as tc:
      for rep in range(reps):
        with ExitStack() as ctx:
            persist = ctx.enter_context(
                tc.tile_pool(name=f"persist{rep}", bufs=1))
            fin_pool = ctx.enter_context(
                tc.tile_pool(name=f"finpool{rep}", bufs=6))

            qkT_sb = persist.tile([P, seq], f16, tag="qkT")
            cqkT_sb = persist.tile([P, seq], f16, tag="cqkT")
            v_sb = persist.tile([P, NT, hpc * vw], f16, tag="v")
            cv_sb = persist.tile([P, NT, hpc * vw], f16, tag="cv")
            wout_sb = persist.tile([P, dim], f16, tag="wout")
            wcout_sb = persist.tile([P, dim], f16, tag="wcout")
            outmT_sb = persist.tile([P, seq], f16, tag="outmT")
            ctxmT_sb = persist.tile([P, seq], f16, tag="ctxmT")
            ones_f = persist.tile([1, dh], f32, tag="onesf",
                                  name=f"ones_f_{rep}")
            nc.vector.memset(ones_f, 1.0)
            ones_r = persist.tile([1, dh], f32r, tag="ones",
                                  name=f"ones_r_{rep}")
            with nc.allow_low_precision(reason="ones constant, exact in f32r"):
                nc.vector.tensor_copy(ones_r, ones_f)
            # preload the Exp activation table off the critical path (the
            # first real exp would otherwise eat a ~1.3us table load)
            warm = persist.tile([1, dh], f16, tag="warm", name=f"warm_{rep}")
            nc.scalar.activation(warm, ones_f, Exp, scale=1.0)

            # ---- load x/context + weights; chunk-group 0 of the
            # projections runs here (8 psum accumulators); chunk-group 1 is
            # sprinkled into phase 0 below through a 4-bank pool.
            xc_pool = ctx.enter_context(
                tc.tile_pool(name=f"xcpool{rep}", bufs=1))
            with tc.tile_pool(name=f"psproj{rep}", bufs=8,
                              space="PSUM") as ps_proj:
                w_sbs = {}
                for name, ap_ in (("wqk", wqk), ("wv", wv), ("wcqk", wcqk),
                                  ("wcv", wcv)):
                    t = xc_pool.tile([P, KT, fpc], f16, tag=name)
                    nc.sync.dma_start(t, ap_.rearrange("(kt p) f -> p kt f",
                                                       p=P))
                    w_sbs[name] = t
                xT_sb = xc_pool.tile([P, KT, seq], f16, tag="xT")
                cT_sb = xc_pool.tile([P, KT, seq], f16, tag="cT")
                xT_v = xT.rearrange("(kt p) i -> p kt i", p=P)
                cT_v = cT.rearrange("(kt p) i -> p kt i", p=P)
                for kt in range(KT):
                    nc.sync.dma_start(xT_sb[:, kt], xT_v[:, kt])
                    nc.sync.dma_start(cT_sb[:, kt], cT_v[:, kt])
                # final-projection weights are needed only in the epilogue:
                # keep them behind the x/context stream in the DMA queue
                nc.sync.dma_start(wout_sb, wout)
                nc.sync.dma_start(wcout_sb, wcout)

                from concourse.masks import make_identity
                ident = persist.tile([P, P], f16, tag="ident")
                make_identity(nc, ident)
                # PE p-state warmup: the array ramps 0.65->2.4 GHz only after
                # sustained execution; burn dummy matmuls into a proj psum
                # tile during the input-DMA wait so the projections start at
                # full clock. The first real matmul's start=True resets psum.
                pe_warm = ps_proj.tile([P, 512], f32, tag="pp",
                                       name=f"pe_warm_{rep}", bufs=8)
                for _ in range(24):
                    nc.tensor.matmul(pe_warm[:, :P], ident, ident,
                                     start=True, stop=True)
                for h in range(hpc):
                    nc.vector.memset(v_sb[:, :, h * vw + dh], 1.0)
                    nc.vector.memset(cv_sb[:, :, h * vw + dh], 1.0)
                vT_tmps = {}
                vT_tmps["wv"] = persist.tile([P, seq], f16, tag="vT_wv",
                                             name=f"vT_wv_{rep}")
                vT_tmps["wcv"] = persist.tile([P, seq], f16, tag="vT_wcv",
                                              name=f"vT_wcv_{rep}")
                # projections: 4 tensors x 4 chunks; two chunk-group passes,
                # 8 psum accumulators live per pass, ktile-major so matmuls
                # chase the input DMAs
                projs = ((xT_sb, "wqk", qkT_sb), (cT_sb, "wcqk", cqkT_sb),
                         (xT_sb, "wv", vT_tmps["wv"]),
                         (cT_sb, "wcv", vT_tmps["wcv"]))
                CPG = NCH // 2                      # chunks per group (2)
                tiles = {}
                for pi in range(4):
                    for cc in range(CPG):
                        tiles[(pi, cc)] = ps_proj.tile(
                            [P, 512], f32, tag="pp",
                            name=f"pp_{rep}_0_{pi}_{cc}")
                for kt in range(KT):
                    for pi, (src_sb, wname, dst) in enumerate(projs):
                        for cc in range(CPG):
                            nc.tensor.matmul(
                                tiles[(pi, cc)], w_sbs[wname][:, kt],
                                src_sb[:, kt, _ts(cc, 512)],
                                start=(kt == 0), stop=(kt == KT - 1))
                for pi, (src_sb, wname, dst) in enumerate(projs):
                    for cc in range(CPG):
                        nc.vector.tensor_copy(dst[:, _ts(cc, 512)],
                                              tiles[(pi, cc)])
            ps_pool = ctx.enter_context(
                tc.tile_pool(name=f"pspool{rep}", bufs=2, space="PSUM"))

            # ---- per-head attention (software-pipelined emission) ----
            e_pool = ctx.enter_context(tc.tile_pool(name=f"epool{rep}",
                                                    bufs=2))
            hg_pool = ctx.enter_context(tc.tile_pool(name=f"hgpool{rep}",
                                                     bufs=2))
            norm_pool = ctx.enter_context(tc.tile_pool(name=f"normpool{rep}",
                                                       bufs=2))

            def cg1_work(ps_proj4):
                """Chunk-group 1 of the projections (seq cols 1024-2047),
                4 accumulators at a time; qk/cqk first so phase 0's later
                sim i-blocks find their lhsT columns ready."""
                groups = (((0, 2), (1, 2), (0, 3), (1, 3)),
                          ((2, 2), (3, 2), (2, 3), (3, 3)))
                for gi, grp in enumerate(groups):
                    gtiles = []
                    for sl, (pi, icx) in enumerate(grp):
                        gtiles.append(ps_proj4.tile(
                            [P, 512], f32, tag="pp4",
                            name=f"pp4_{rep}_{gi}_{sl}"))
                    for kt in range(KT):
                        for sl, (pi, icx) in enumerate(grp):
                            src_sb, wname, dst = projs[pi]
                            nc.tensor.matmul(
                                gtiles[sl], w_sbs[wname][:, kt],
                                src_sb[:, kt, _ts(icx, 512)],
                                start=(kt == 0), stop=(kt == KT - 1))
                            yield
                    for sl, (pi, icx) in enumerate(grp):
                        src_sb, wname, dst = projs[pi]
                        nc.vector.tensor_copy(dst[:, _ts(icx, 512)],
                                              gtiles[sl])
                        yield

            def vcv_transpose_work():
                for wname, dst in (("wv", v_sb), ("wcv", cv_sb)):
                    vT_tmp = vT_tmps[wname]
                    for ibg in range(NT // 4):
                        pst = ps_pool.tile([P, 1024], f32, tag="ps")
                        pst16 = pst.bitcast(f16)
                        for k in range(4):
                            nc.tensor.transpose(pst16[:, _ts(k, P)],
                                                vT_tmp[:, _ts(ibg * 4 + k, P)],
                                                ident)
                            yield
                        pstv = pst16[:, :4 * P].rearrange("p (k f) -> p k f",
                                                          k=4)
                        for h in range(hpc):
                            nc.vector.tensor_copy(
                                dst[:, ibg * 4:(ibg + 1) * 4,
                                    h * vw:h * vw + dh],
                                pstv[:, :, h * dh:(h + 1) * dh])
                        yield

            def acc_work(h, half, E_half, side):
                """Accumulate H^T (side 0) / G^T (side 1) for this
                (head, seq-half) from its exp-tile stream + normalize."""
                hs = slice(h * dh, (h + 1) * dh)
                va = slice(h * vw, h * vw + vw)
                vec_sb = v_sb if side == 0 else cv_sb
                dstmT = ctxmT_sb if side == 0 else outmT_sb
                psH = ps_acc.tile([vw, J2], f32, tag="acc")
                hT = hg_pool.tile([vw, J2], f16, tag="ht")
                rcs_r = norm_pool.tile([1, J2], f32r, tag="rc",
                                       name=f"rcs_r_{rep}")
                for jcc in range(HPT):
                    jsl_l = _ts(jcc, 512)
                    for it in range(NT):
                        nc.tensor.matmul(psH[:, jsl_l], vec_sb[:, it, va],
                                         E_half[:, it, jsl_l],
                                         start=(it == 0), stop=(it == NT - 1))
                        yield
                    nc.vector.tensor_copy(hT[:, jsl_l], psH[:, jsl_l])
                    with nc.allow_low_precision(reason="softmax sums O(2e3); f32r rounding is ~1e-7 rel"):
                        nc.vector.reciprocal(rcs_r[:, jsl_l],
                                             hT[dh:dh + 1, jsl_l])
                    jsl_g = _ts(half * HPT + jcc, 512)
                    bc = hg_pool.tile([dh, 512], f32r, tag="bc",
                                      name=f"bc_{rep}")
                    nc.gpsimd.partition_broadcast(bc, rcs_r[0:1, jsl_l])
                    nc.vector.tensor_mul(dstmT[hs, jsl_g], hT[0:dh, jsl_l],
                                         bc)
                    yield

            def final_work(mT, w_sb, odram, ib_lo, ib_hi, tail=False):
                for ib in range(ib_lo, ib_hi):
                    pso = ps_pool.tile([P, 1024], f32, tag="ps")
                    for oc in range(OCH):
                        nc.tensor.matmul(pso[:, _ts(oc, 512)],
                                         mT[:, _ts(ib, P)],
                                         w_sb[:, _ts(oc, 512)],
                                         start=True, stop=True)
                        yield
                    osb = fin_pool.tile([P, dim], f16, tag="osb")
                    if tail and ib % 2 == 0:
                        # past the last exp the ACT engine is idle; splitting
                        # the copies between ACT and DVE halves the tail pace
                        nc.scalar.copy(osb, pso[:, :dim])
                    else:
                        nc.vector.tensor_copy(osb, pso[:, :dim])
                    nc.sync.dma_start(odram[:, ib, :], osb)
                    yield

            pending = deque()

            def sprinkle(n):
                done = 0
                while pending and done < n:
                    try:
                        next(pending[0])
                        done += 1
                    except StopIteration:
                        pending.popleft()

            # side-major phase order: all ctx-side (E) phases, then all
            # out-side (ET) phases.
            # side 0: lhsT = qkT i-block, rhs = cqkT j-cols -> E[i,j]
            # side 1: lhsT = cqkT j-block, rhs = qkT i-cols -> ET[j,i]
            phases = ([(h, 0, half) for h in range(hpc)
                       for half in range(N2CH)] +
                      [(h, 1, half) for h in range(hpc)
                       for half in range(N2CH)])
            NPH = len(phases)

            def emit_phase(pi_, spr):
                h, side, half = phases[pi_]
                hs = slice(h * dh, (h + 1) * dh)
                lhsT_sb, rhs_sb = ((qkT_sb, cqkT_sb) if side == 0
                                   else (cqkT_sb, qkT_sb))
                E_half = e_pool.tile([P, NT, J2], f16, tag="e",
                                     name=f"E_{rep}_{pi_}")
                for it in range(NT):
                    ps = ps_pool.tile([P, 1024], f32, tag="ps")
                    for hlf in range(HPT):
                        js = _ts(half * HPT + hlf, 512)
                        nc.tensor.matmul(ps[:, _ts(hlf, 512)],
                                         lhsT_sb[hs, _ts(it, P)],
                                         rhs_sb[hs, js],
                                         start=True, stop=True)
                    nc.scalar.activation(E_half[:, it, :], ps[:, :J2],
                                         Exp, scale=SCALE)
                    sprinkle(spr)
                if stage not in ('e0', 'e'):
                    pending.append(acc_work(h, half, E_half, side))
                    # ctx final: after the LAST ctx-side accumulation has been
                    # emitted (start of out-side phases). out final half 0:
                    # after both heads' out-side half-0 accumulations.
                    if stage == 'full':
                        if pi_ == NPH // 2:
                            # first out-side phase just emitted; the last
                            # ctx-side acc_work is ahead of us in the FIFO,
                            # so ctx final emits strictly after it.
                            pending.append(final_work(ctxmT_sb, wcout_sb,
                                                      ctx_view, 0, NT))
                        if pi_ == NPH - 1:
                            # last phase emitted; out-side half-0 accs (both
                            # heads) are ahead in the FIFO. out final for the
                            # first half of i rides under this phase's drain.
                            pending.append(final_work(outmT_sb, wout_sb,
                                                      out_view, 0, NT // 2,
                                                      tail=True))
                if SEQ_MODE:
                    while pending:
                        sprinkle(1 << 30)

            # phase 0 runs while chunk-group 1 of the projections drains
            # through a coexisting 4-bank psum pool; ps_acc (needed from
            # phase 1) opens only after that pool closes.
            with tc.tile_pool(name=f"psproj4{rep}", bufs=4,
                              space="PSUM") as ps_proj4:
                cg1 = cg1_work(ps_proj4)
                pending.append(cg1)
                pending.append(vcv_transpose_work())
                emit_phase(0, 5)
                while pending and pending[0] is cg1:
                    try:
                        next(cg1)
                    except StopIteration:
                        pending.popleft()
            ps_acc = ctx.enter_context(
                tc.tile_pool(name=f"psacc{rep}", bufs=2, space="PSUM"))
            for pi_ in range(1, NPH):
                spr = 5 if pi_ == 1 else (3 if pi_ < NPH // 2 else 4)
                emit_phase(pi_, spr)

            if stage != 'full':
                while pending:
                    try:
                        next(pending[0])
                    except StopIteration:
                        pending.popleft()
                dummy = fin_pool.tile([P, dim], f16, tag="osb",
                                      name=f"dummy_{rep}")
                nc.vector.memset(outmT_sb, 0.0)
                nc.vector.memset(ctxmT_sb, 0.0)
                nc.vector.memset(dummy, 0.0)
                nc.sync.dma_start(out_view[:, 0, :], dummy)
                nc.sync.dma_start(ctx_view[:, 0, :], dummy)
                while pending:
                    pending.popleft()
            else:
                # round-robin drain: overlaps the last accumulation stream
                # with the already-enabled final projections
                while pending:
                    try:
                        next(pending[0])
                        pending.rotate(-1)
                    except StopIteration:
                        pending.popleft()
                # out-side final, second half (needs the very last acc_work)
                for _ in final_work(outmT_sb, wout_sb, out_view,
                                    NT // 2, NT, tail=True):
                    pass

    nc.compile()
    return nc


_NC_CACHE = {}


def _get_nc():
    if "nc" not in _NC_CACHE:
        _NC_CACHE["nc"] = build_bass()
    return _NC_CACHE["nc"]


def make_in_maps(x, context, W_qk, W_cqk, W_v, W_cv):
    f16 = np.float16
    xT = np.ascontiguousarray(np.asarray(x, np.float32)[0].T).astype(f16)
    cT = np.ascontiguousarray(np.asarray(context, np.float32)[0].T).astype(f16)
    in_maps = []
    for c in range(N_CORES):
        cs = _ts(c, FPC)
        in_maps.append({
            "xT": xT,
            "cT": cT,
            "wqk": np.ascontiguousarray(np.asarray(W_qk)[:, cs]).astype(f16),
            "wv": np.ascontiguousarray(np.asarray(W_v)[:, cs]).astype(f16),
            "wcqk": np.ascontiguousarray(np.asarray(W_cqk)[:, cs]).astype(f16),
            "wcv": np.ascontiguousarray(np.asarray(W_cv)[:, cs]).astype(f16),
        })
    return in_maps


def add_weight_slices(in_maps, W_out, W_cout):
    f16 = np.float16
    for c in range(N_CORES):
        rs = _ts(c, FPC)
        in_maps[c]["wout"] = np.ascontiguousarray(np.asarray(W_out)[rs, :]).astype(f16)
        in_maps[c]["wcout"] = np.ascontiguousarray(np.asarray(W_cout)[rs, :]).astype(f16)
    return in_maps


def kernel(x, context, W_qk, W_cqk, W_v, W_cv, W_out, b_out, W_cout, b_cout):
    from concourse.bass_utils import run_bass_kernel_spmd

    nc = _get_nc()
    in_maps = make_in_maps(x, context, W_qk, W_cqk, W_v, W_cv)
    add_weight_slices(in_maps, W_out, W_cout)

    res = run_bass_kernel_spmd(nc, in_maps, core_ids=list(range(N_CORES)))

    out = np.zeros((SEQ, DIM), np.float32)
    ctx_out = np.zeros((SEQ, DIM), np.float32)
    for r in res.results:
        out += r["out_p"].astype(np.float32)
        ctx_out += r["ctx_p"].astype(np.float32)
    out += np.asarray(b_out, np.float32)
    ctx_out += np.asarray(b_cout, np.float32)
    return (out[None], ctx_out[None])
